# revision 9
# baseline (speedup 1.0000x reference)
"""Trainium2 Bass kernel for CommunityPassing (segment mean + gather).

Algorithm (8 NeuronCores, data-parallel over nodes):
  host: shard x/community over 8 cores along the node axis; within each
        shard, stably sort node indices by community id and pack them into
        128-row tiles grouped by community "chunk" (128 communities per
        chunk, 8 chunks for 1000 communities). Pad each (core, chunk)
        block to a shared tile count so all cores run one SPMD program.
        Cast x to bf16 and lay tiles out in 16-tile blocks with
        per-partition-contiguous 8KB rows for full-rate DMA.
  dev:  phase 1 - stream sorted x blocks; per tile build a one-hot
        B[node, local_comm] with a DVE is_equal; matmul B^T @ x_tile
        accumulating in PSUM per community chunk -> per-core partial sums
        (bf16).  AllReduce the sums in four 128KB quarters (chunk pairs)
        so the means arrive progressively and the serialized collective
        chain overlaps the phase-2 stream despite ~40us core-launch skew.
        Scale by host-computed 1/count -> community-mean table (bf16).
        phase 2 - per 4-tile group, broadcast the group's local ids
        across partitions with a K=1 matmul (ones^T @ locid_row), build
        B^T directly with one is_equal against the partition index, then
        per tile matmul (lhsT=B^T) @ mean_chunk -> out tile.  The B^T
        builds have no collective dependency and run ahead of the gated
        matmul+copy+store stream (RUNAHEAD ring).  PSUM->SBUF copies are
        split between the Scalar and Vector engines; 16-tile 1MB blocks
        are stored with per-partition-contiguous rows.
  host: unpack blocks, scatter rows back to original order, cast to f32.
"""

import os
import sys

import numpy as np

for _p in ("/opt/trn_rl_repo", "/opt/pypackages"):
    if _p not in sys.path and os.path.isdir(_p):
        sys.path.append(_p)

# Problem constants (hardcoded per the task contract).
N_FULL = 500000
F = 256
NUM_COMMS = 1000
EPS = 1e-12
M = 8               # cores
P = 128             # partitions
NC_CHUNKS = 8       # community chunks of 128 (8*128 = 1024 >= 1000)
BT = 16             # tiles per x/out DMA block (8KB per partition = 1MB)
G = 4               # tiles per phase-2 B^T build group
LAG_TILES = 150  # covers ~40us core-launch skew + AR latency
RUNAHEAD = 20    # B^T build groups allowed ahead of the matmul stream     # phase-1 tiles between AR issue and first dependent p2 op

# Stash of the most recent run's BassKernelResults (for test harnesses).
LAST_RESULTS = None


def _host_prep(x, community):
    """Build per-core device inputs. Returns (in_maps, plan)."""
    import ml_dtypes

    bf16 = ml_dtypes.bfloat16
    x = np.ascontiguousarray(np.asarray(x, dtype=np.float32))
    community = np.asarray(community).astype(np.int64)
    n = x.shape[0]
    assert n % M == 0
    nl = n // M

    comm_sh = community.reshape(M, nl)
    perms = np.argsort(comm_sh, axis=1, kind="stable")
    comm_sorted = np.take_along_axis(comm_sh, perms, axis=1)

    # per (core, chunk) node counts
    chunk_ids = comm_sorted >> 7  # // 128
    cnts = np.zeros((M, NC_CHUNKS), dtype=np.int64)
    for m in range(M):
        bc = np.bincount(chunk_ids[m], minlength=NC_CHUNKS)
        cnts[m] = bc[:NC_CHUNKS]
    t_k = np.maximum(1, -(-cnts.max(axis=0) // P))  # ceil, shared by all cores
    # pad the total tile count to a multiple of BT (extend the last chunk)
    t_total = int(t_k.sum())
    pad = (-t_total) % BT
    t_k[-1] += pad
    t_total += pad
    chunk_of_tile = np.repeat(np.arange(NC_CHUNKS), t_k)
    tile_off = np.concatenate([[0], np.cumsum(t_k)])  # tile index base per chunk

    # counts -> 1/max(cnt, eps), [p, k] layout (community id = k*128 + p)
    cnt_full = np.bincount(community, minlength=NUM_COMMS).astype(np.float32)
    inv_pad = np.zeros((NC_CHUNKS * P,), np.float32)
    inv_pad[:NUM_COMMS] = 1.0 / np.maximum(cnt_full, np.float32(EPS))
    invc = np.ascontiguousarray(inv_pad.reshape(NC_CHUNKS, P).T)  # [128, 8]

    iota_t = np.ascontiguousarray(
        np.tile(np.arange(P, dtype=np.float32), (P, 1))
    ).astype(bf16)  # [128, 128], each row 0..127
    iota_col = np.arange(P, dtype=np.float32).reshape(P, 1)  # [128, 1]
    ones_t = np.ones((P, P), dtype=bf16)
    iota4 = np.ascontiguousarray(np.tile(iota_t, (1, G)))  # [128, G*128]

    nblk = t_total // BT
    ngrp = t_total // G
    roww = -(-ngrp // 3) * (G * P)  # locid_row cols per partition-slot

    in_maps = []
    origs = []
    for m in range(M):
        x_m = x[m * nl : (m + 1) * nl]
        xs = np.zeros((t_total * P, F), dtype=bf16)
        locid = np.full((t_total * P,), -1.0, dtype=np.float32)
        orig = np.full((t_total * P,), -1, dtype=np.int64)
        start = 0
        for k in range(NC_CHUNKS):
            c = int(cnts[m, k])
            row = int(tile_off[k]) * P
            sel = perms[m, start : start + c]
            xs[row : row + c] = x_m[sel]
            orig[row : row + c] = sel
            locid[row : row + c] = comm_sorted[m, start : start + c] - k * P
            start += c
        origs.append(orig)

        # blocked x layout: [nblk, P, BT*F] with 8KB contiguous per partition
        xs_blk = np.ascontiguousarray(
            xs.reshape(nblk, BT, P, F).transpose(0, 2, 1, 3)
        ).reshape(nblk * P, BT * F)
        # per-tile local ids, [128, T] (partition = node-in-tile)
        locid_t = np.ascontiguousarray(locid.reshape(t_total, P).T)
        # groups of G tiles packed at partitions {0,32,64,96} for the
        # K=1 broadcast matmul (tile_position constraint)
        locid_row = np.zeros((P, roww), dtype=bf16)
        lr = locid.reshape(ngrp, G * P).astype(bf16)
        for q in range(ngrp):
            part = (q % 3) * 32
            c0 = (q // 3) * (G * P)
            locid_row[part, c0 : c0 + G * P] = lr[q]

        in_maps.append(
            {
                "xs": xs_blk,
                "locid": locid_t,
                "locid_row": locid_row,
                "iota": iota_t,
                "iota4": iota4,
                "iota_col": iota_col,
                "ones": ones_t,
                "invc": invc,
            }
        )

    plan = {
        "nl": nl,
        "t_total": t_total,
        "chunk_of_tile": [int(v) for v in chunk_of_tile],
        "tile_off": [int(v) for v in tile_off],
        "origs": origs,
        "roww": roww,
    }
    return in_maps, plan


def _build_program(plan, use_collective=True):
    from concourse import bacc, mybir, tile

    t_total = plan["t_total"]
    chunk_of_tile = plan["chunk_of_tile"]
    tile_off = plan["tile_off"]
    roww = plan["roww"]
    nblk = t_total // BT
    ngrp = t_total // G
    QW = 2 * F  # 2 chunks of the community table per AR quarter

    dt = mybir.dt
    nc = bacc.Bacc("TRN2", target_bir_lowering=False, debug=False, num_devices=M)

    xs = nc.dram_tensor("xs", [nblk * P, BT * F], dt.bfloat16, kind="ExternalInput")
    locid = nc.dram_tensor("locid", [P, t_total], dt.float32, kind="ExternalInput")
    locid_row = nc.dram_tensor(
        "locid_row", [P, roww], dt.bfloat16, kind="ExternalInput"
    )
    iota = nc.dram_tensor("iota", [P, P], dt.bfloat16, kind="ExternalInput")
    iota4 = nc.dram_tensor("iota4", [P, G * P], dt.bfloat16, kind="ExternalInput")
    iota_col = nc.dram_tensor("iota_col", [P, 1], dt.float32, kind="ExternalInput")
    ones = nc.dram_tensor("ones", [P, P], dt.bfloat16, kind="ExternalInput")
    invc = nc.dram_tensor("invc", [P, NC_CHUNKS], dt.float32, kind="ExternalInput")
    out = nc.dram_tensor("out", [nblk * P, BT * F], dt.bfloat16, kind="ExternalOutput")

    xs_view = xs.ap().rearrange("(b p) w -> b p w", p=P)  # [nblk, 128, BT*F]
    out_view = out.ap().rearrange("(b p) w -> b p w", p=P)

    with tile.TileContext(nc) as tc:
        with (
            tc.tile_pool(name="const", bufs=1) as constp,
            tc.tile_pool(name="acc", bufs=1) as accp,
            tc.tile_pool(name="xsp", bufs=3) as xsp,
            tc.tile_pool(name="bp", bufs=4) as bp,
            tc.tile_pool(name="btp", bufs=22) as btp,
            tc.tile_pool(name="outp", bufs=3) as outp,
            tc.tile_pool(name="ps1", bufs=2, space="PSUM") as ps1,
            tc.tile_pool(name="psB", bufs=2, space="PSUM") as psB,
            tc.tile_pool(name="psO", bufs=3, space="PSUM") as psO,
            tc.tile_pool(name="dram", bufs=1, space="DRAM") as dramp,
        ):
            iota_t = constp.tile([P, P], dt.bfloat16)
            nc.sync.dma_start(out=iota_t[:], in_=iota.ap())
            iota4_t = constp.tile([P, G * P], dt.bfloat16)
            nc.sync.dma_start(out=iota4_t[:], in_=iota4.ap())
            iota_col_t = constp.tile([P, 1], dt.float32)
            nc.sync.dma_start(out=iota_col_t[:], in_=iota_col.ap())
            ones_t = constp.tile([P, P], dt.bfloat16)
            nc.sync.dma_start(out=ones_t[:], in_=ones.ap())
            locid_t = constp.tile([P, t_total], dt.float32)
            nc.sync.dma_start(out=locid_t[:], in_=locid.ap())
            locid_row_t = constp.tile([P, roww], dt.bfloat16)
            nc.sync.dma_start(out=locid_row_t[:], in_=locid_row.ap())
            invc_t = constp.tile([P, NC_CHUNKS], dt.float32)
            nc.sync.dma_start(out=invc_t[:], in_=invc.ap())

            # per-core partial community sums (bf16) and the mean table
            comm_sum = accp.tile([P, NC_CHUNKS * F], dt.bfloat16)
            mean_sb = accp.tile([P, NC_CHUNKS * F], dt.bfloat16)

            ar_bufs = []
            for h in range(4):
                ar_in = dramp.tile([P, QW], dt.bfloat16, name=f"ar_in{h}")
                ar_out = dramp.tile([P, QW], dt.bfloat16, name=f"ar_out{h}")
                ar_bufs.append((ar_in, ar_out))

            def emit_ar_quarter(h):
                ar_in, ar_out = ar_bufs[h]
                o = h * QW
                nc.sync.dma_start(out=ar_in, in_=comm_sum[:, o : o + QW])
                if use_collective:
                    nc.gpsimd.collective_compute(
                        "AllReduce",
                        mybir.AluOpType.add,
                        replica_groups=[list(range(M))],
                        ins=[ar_in.opt()],
                        outs=[ar_out.opt()],
                    )
                else:
                    nc.sync.dma_start(out=ar_out, in_=ar_in)
                nc.sync.dma_start(out=mean_sb[:, o : o + QW], in_=ar_out)
                for k in range(2 * h, 2 * h + 2):
                    nc.vector.tensor_scalar(
                        mean_sb[:, k * F : (k + 1) * F],
                        mean_sb[:, k * F : (k + 1) * F],
                        invc_t[:, k : k + 1],
                        None,
                        mybir.AluOpType.mult,
                    )

            # ---- phase-2 emission: B^T build stream + matmul stream ----
            outsb = [None]
            bt_aps = {}

            def emit_bt_build(q):
                bpart = (q % 3) * 32
                c0 = (q // 3) * (G * P)
                bc = psB.tile([P, G * P], dt.float32, tag="bc")
                nc.tensor.matmul(
                    bc[:],
                    lhsT=ones_t[bpart : bpart + 1, :],
                    rhs=locid_row_t[bpart : bpart + 1, c0 : c0 + G * P],
                    start=True,
                    stop=True,
                )
                btq = btp.tile([P, G * P], dt.bfloat16, tag="bt", name="btq")
                nc.vector.tensor_scalar(
                    btq[:],
                    bc[:],
                    iota_col_t[:, 0:1],
                    None,
                    mybir.AluOpType.is_equal,
                )
                bt_aps[q] = btq

            def emit_mm_part(q):
                t0 = q * G
                btq = bt_aps.pop(q)
                po = None
                for j2 in range(G):
                    t = t0 + j2
                    k = chunk_of_tile[t]
                    jb = t % BT
                    if jb == 0:
                        outsb[0] = outp.tile(
                            [P, BT * F], dt.bfloat16, tag="osb", name="osb"
                        )
                    if j2 % 2 == 0:
                        po = psO.tile([P, 2 * F], dt.float32, tag="po")
                    nc.tensor.matmul(
                        po[:, (j2 % 2) * F : (j2 % 2 + 1) * F],
                        lhsT=btq[:, j2 * P : (j2 + 1) * P],
                        rhs=mean_sb[:, k * F : (k + 1) * F],
                        start=True,
                        stop=True,
                    )
                    if j2 % 2 == 1:
                        if j2 == 1 and q % 2 == 1:
                            nc.vector.tensor_copy(
                                out=outsb[0][:, (jb - 1) * F : (jb + 1) * F],
                                in_=po[:],
                            )
                        else:
                            nc.scalar.copy(
                                out=outsb[0][:, (jb - 1) * F : (jb + 1) * F],
                                in_=po[:],
                            )
                    if jb == BT - 1:
                        b = t // BT
                        nc.sync.dma_start(
                            out=out_view[b], in_=outsb[0][:]
                        )

            # ---- main emission loop: phase 1 with interleaved phase 2 ----
            xsb = None
            ps = None
            b4 = None
            bt_built = 0
            mm_done = 0
            ar_emit_tile = {}

            for t in range(t_total):
                if t % BT == 0:
                    b = t // BT
                    xsb = xsp.tile([P, BT * F], dt.bfloat16, tag="xsb")
                    nc.sync.dma_start(out=xsb[:], in_=xs_view[b])
                if t % G == 0:
                    # batched one-hot build for G phase-1 tiles in one DVE op
                    b4 = bp.tile([P, G * P], dt.bfloat16, tag="b")
                    nc.vector.tensor_tensor(
                        out=b4[:].rearrange("p (g j) -> p g j", g=G),
                        in0=iota4_t[:].rearrange("p (g j) -> p g j", g=G),
                        in1=locid_t[:, t : t + G]
                        .unsqueeze(2)
                        .broadcast_to([P, G, P]),
                        op=mybir.AluOpType.is_equal,
                    )
                k = chunk_of_tile[t]
                first = t == tile_off[k]
                last = t == tile_off[k + 1] - 1
                if first:
                    ps = ps1.tile([P, F], dt.float32, tag="ps")
                j = t % BT
                jg = t % G
                nc.tensor.matmul(
                    ps[:],
                    lhsT=b4[:, jg * P : (jg + 1) * P],
                    rhs=xsb[:, j * F : (j + 1) * F],
                    start=first,
                    stop=last,
                )
                if last:
                    nc.vector.tensor_copy(
                        out=comm_sum[:, k * F : (k + 1) * F], in_=ps[:]
                    )
                    if k % 2 == 1:
                        emit_ar_quarter(k // 2)
                        ar_emit_tile[k // 2] = t
                # phase-2 streams, paced 1-per-G phase-1 tiles:
                # B^T builds have no AR dependency and run ahead; the
                # matmul+copy+store stream is gated on its AR half
                if t % G == G - 1:
                    if bt_built < ngrp and bt_built < mm_done + RUNAHEAD:
                        emit_bt_build(bt_built)
                        bt_built += 1
                    if mm_done < bt_built:
                        h = chunk_of_tile[mm_done * G + G - 1] // 2
                        if h in ar_emit_tile and t >= ar_emit_tile[h] + LAG_TILES:
                            emit_mm_part(mm_done)
                            mm_done += 1

            # tail: alternate remaining builds and matmul parts
            while mm_done < ngrp:
                if bt_built < ngrp and bt_built < mm_done + RUNAHEAD:
                    emit_bt_build(bt_built)
                    bt_built += 1
                else:
                    emit_mm_part(mm_done)
                    mm_done += 1

    nc.compile()
    return nc


def kernel(x, community):
    global LAST_RESULTS
    from concourse.bass_utils import run_bass_kernel_spmd

    in_maps, plan = _host_prep(x, community)
    nc = _build_program(plan)
    res = run_bass_kernel_spmd(nc, in_maps, core_ids=list(range(M)))
    LAST_RESULTS = res
    nl = plan["nl"]
    t_total = plan["t_total"]
    nblk = t_total // BT
    outs = []
    for m in range(M):
        out_blk = np.asarray(res.results[m]["out"])  # [nblk*P, BT*F] bf16
        out_sorted = (
            out_blk.reshape(nblk, P, BT, F)
            .transpose(0, 2, 1, 3)
            .reshape(t_total * P, F)
        )
        orig = plan["origs"][m]
        valid = orig >= 0
        out_m = np.empty((nl, F), dtype=np.float32)
        out_m[orig[valid]] = out_sorted[valid]
        outs.append(out_m)
    return np.concatenate(outs, axis=0)


# revision 12
# speedup vs baseline: 1.1051x; 1.1051x over previous
"""Trainium2 Bass kernel for CommunityPassing (segment mean + gather).

Algorithm (8 NeuronCores, data-parallel over nodes):
  host: shard x/community over 8 cores along the node axis; within each
        shard, stably sort node indices by community id and pack them into
        128-row tiles grouped by community "chunk" (128 communities per
        chunk, 8 chunks for 1000 communities). Pad each (core, chunk)
        block to a shared tile count so all cores run one SPMD program.
        Cast x to bf16 and lay tiles out in 16-tile blocks with
        per-partition-contiguous 8KB rows for full-rate DMA.
  dev:  phase 1 - stream sorted x blocks; per tile build a one-hot
        B[node, local_comm] with a DVE is_equal; matmul B^T @ x_tile
        accumulating in PSUM per community chunk -> per-core partial sums
        (bf16).  AllReduce the sums in four 128KB quarters (chunk pairs)
        so the means arrive progressively and the serialized collective
        chain overlaps the phase-2 stream despite ~40us core-launch skew.
        Scale by host-computed 1/count -> community-mean table (bf16).
        phase 2 - per 4-tile group, broadcast the group's local ids
        across partitions with a K=1 matmul (ones^T @ locid_row), build
        B^T directly with one is_equal against the partition index, then
        per tile matmul (lhsT=B^T) @ mean_chunk -> out tile.  The B^T
        builds have no collective dependency and run ahead of the gated
        matmul+copy+store stream (RUNAHEAD ring).  PSUM->SBUF copies are
        split between the Scalar and Vector engines; 16-tile 1MB blocks
        are stored with per-partition-contiguous rows.
  host: unpack blocks, scatter rows back to original order, cast to f32.
"""

import os
import sys

import numpy as np

for _p in ("/opt/trn_rl_repo", "/opt/pypackages"):
    if _p not in sys.path and os.path.isdir(_p):
        sys.path.append(_p)

# Problem constants (hardcoded per the task contract).
N_FULL = 500000
F = 256
NUM_COMMS = 1000
EPS = 1e-12
M = 8               # cores
P = 128             # partitions
NC_CHUNKS = 8       # community chunks of 128 (8*128 = 1024 >= 1000)
BT = 16             # tiles per x/out DMA block (8KB per partition = 1MB)
G = 4               # tiles per phase-2 B^T build group
LAG_TILES = 150  # covers ~40us core-launch skew + AR latency
RUNAHEAD = 20    # B^T build groups allowed ahead of the matmul stream     # phase-1 tiles between AR issue and first dependent p2 op

# Stash of the most recent run's BassKernelResults (for test harnesses).
LAST_RESULTS = None


def _host_prep(x, community):
    """Build per-core device inputs. Returns (in_maps, plan)."""
    import ml_dtypes

    bf16 = ml_dtypes.bfloat16
    x = np.ascontiguousarray(np.asarray(x, dtype=np.float32))
    community = np.asarray(community).astype(np.int64)
    n = x.shape[0]
    assert n % M == 0
    nl = n // M

    comm_sh = community.reshape(M, nl)
    perms = np.argsort(comm_sh, axis=1, kind="stable")
    comm_sorted = np.take_along_axis(comm_sh, perms, axis=1)

    # per (core, chunk) node counts
    chunk_ids = comm_sorted >> 7  # // 128
    cnts = np.zeros((M, NC_CHUNKS), dtype=np.int64)
    for m in range(M):
        bc = np.bincount(chunk_ids[m], minlength=NC_CHUNKS)
        cnts[m] = bc[:NC_CHUNKS]
    t_k = np.maximum(1, -(-cnts.max(axis=0) // P))  # ceil, shared by all cores
    # pad the total tile count to a multiple of BT (extend the last chunk)
    t_total = int(t_k.sum())
    pad = (-t_total) % BT
    t_k[-1] += pad
    t_total += pad
    chunk_of_tile = np.repeat(np.arange(NC_CHUNKS), t_k)
    tile_off = np.concatenate([[0], np.cumsum(t_k)])  # tile index base per chunk

    # counts -> 1/max(cnt, eps), [p, k] layout (community id = k*128 + p)
    cnt_full = np.bincount(community, minlength=NUM_COMMS).astype(np.float32)
    inv_pad = np.zeros((NC_CHUNKS * P,), np.float32)
    inv_pad[:NUM_COMMS] = 1.0 / np.maximum(cnt_full, np.float32(EPS))
    invc = np.ascontiguousarray(inv_pad.reshape(NC_CHUNKS, P).T)  # [128, 8]

    iota_t = np.ascontiguousarray(
        np.tile(np.arange(P, dtype=np.float32), (P, 1))
    ).astype(bf16)  # [128, 128], each row 0..127
    iota_col = np.arange(P, dtype=np.float32).reshape(P, 1)  # [128, 1]
    ones_t = np.ones((P, P), dtype=bf16)
    iota4 = np.ascontiguousarray(np.tile(iota_t, (1, G)))  # [128, G*128]

    nblk = t_total // BT
    ngrp = t_total // G
    roww = -(-ngrp // 3) * (G * P)  # locid_row cols per partition-slot

    in_maps = []
    origs = []
    for m in range(M):
        x_m = x[m * nl : (m + 1) * nl]
        xs = np.zeros((t_total * P, F), dtype=bf16)
        locid = np.full((t_total * P,), -1.0, dtype=np.float32)
        orig = np.full((t_total * P,), -1, dtype=np.int64)
        start = 0
        for k in range(NC_CHUNKS):
            c = int(cnts[m, k])
            row = int(tile_off[k]) * P
            sel = perms[m, start : start + c]
            xs[row : row + c] = x_m[sel]
            orig[row : row + c] = sel
            locid[row : row + c] = comm_sorted[m, start : start + c] - k * P
            start += c
        origs.append(orig)

        # blocked x layout: [nblk, P, BT*F] with 8KB contiguous per partition
        xs_blk = np.ascontiguousarray(
            xs.reshape(nblk, BT, P, F).transpose(0, 2, 1, 3)
        ).reshape(nblk * P, BT * F)
        # per-tile local ids, [128, T] (partition = node-in-tile)
        locid_t = np.ascontiguousarray(locid.reshape(t_total, P).T)
        # groups of G tiles packed at partitions {0,32,64,96} for the
        # K=1 broadcast matmul (tile_position constraint)
        locid_row = np.zeros((P, roww), dtype=bf16)
        lr = locid.reshape(ngrp, G * P).astype(bf16)
        for q in range(ngrp):
            part = (q % 3) * 32
            c0 = (q // 3) * (G * P)
            locid_row[part, c0 : c0 + G * P] = lr[q]

        in_maps.append(
            {
                "xs": xs_blk,
                "locid": locid_t,
                "locid_row": locid_row,
                "iota": iota_t,
                "iota4": iota4,
                "iota_col": iota_col,
                "ones": ones_t,
                "invc": invc,
            }
        )

    plan = {
        "nl": nl,
        "t_total": t_total,
        "chunk_of_tile": [int(v) for v in chunk_of_tile],
        "tile_off": [int(v) for v in tile_off],
        "origs": origs,
        "roww": roww,
    }
    return in_maps, plan


def _build_program(plan, use_collective=True):
    from concourse import bacc, mybir, tile

    t_total = plan["t_total"]
    chunk_of_tile = plan["chunk_of_tile"]
    tile_off = plan["tile_off"]
    roww = plan["roww"]
    nblk = t_total // BT
    ngrp = t_total // G
    QW = 2 * F  # 2 chunks of the community table per AR quarter

    dt = mybir.dt
    nc = bacc.Bacc("TRN2", target_bir_lowering=False, debug=False, num_devices=M)

    xs = nc.dram_tensor("xs", [nblk * P, BT * F], dt.bfloat16, kind="ExternalInput")
    locid = nc.dram_tensor("locid", [P, t_total], dt.float32, kind="ExternalInput")
    locid_row = nc.dram_tensor(
        "locid_row", [P, roww], dt.bfloat16, kind="ExternalInput"
    )
    iota = nc.dram_tensor("iota", [P, P], dt.bfloat16, kind="ExternalInput")
    iota4 = nc.dram_tensor("iota4", [P, G * P], dt.bfloat16, kind="ExternalInput")
    iota_col = nc.dram_tensor("iota_col", [P, 1], dt.float32, kind="ExternalInput")
    ones = nc.dram_tensor("ones", [P, P], dt.bfloat16, kind="ExternalInput")
    invc = nc.dram_tensor("invc", [P, NC_CHUNKS], dt.float32, kind="ExternalInput")
    out = nc.dram_tensor("out", [nblk * P, BT * F], dt.bfloat16, kind="ExternalOutput")

    xs_view = xs.ap().rearrange("(b p) w -> b p w", p=P)  # [nblk, 128, BT*F]
    out_view = out.ap().rearrange("(b p) w -> b p w", p=P)

    with tile.TileContext(nc) as tc:
        with (
            tc.tile_pool(name="const", bufs=1) as constp,
            tc.tile_pool(name="acc", bufs=1) as accp,
            tc.tile_pool(name="xsp", bufs=3) as xsp,
            tc.tile_pool(name="bp", bufs=4) as bp,
            tc.tile_pool(name="btp", bufs=22) as btp,
            tc.tile_pool(name="outp", bufs=3) as outp,
            tc.tile_pool(name="ps1", bufs=2, space="PSUM") as ps1,
            tc.tile_pool(name="psB", bufs=2, space="PSUM") as psB,
            tc.tile_pool(name="psO", bufs=3, space="PSUM") as psO,
            tc.tile_pool(name="dram", bufs=1, space="DRAM") as dramp,
        ):
            iota_t = constp.tile([P, P], dt.bfloat16)
            nc.sync.dma_start(out=iota_t[:], in_=iota.ap())
            iota4_t = constp.tile([P, G * P], dt.bfloat16)
            nc.sync.dma_start(out=iota4_t[:], in_=iota4.ap())
            iota_col_t = constp.tile([P, 1], dt.float32)
            nc.sync.dma_start(out=iota_col_t[:], in_=iota_col.ap())
            ones_t = constp.tile([P, P], dt.bfloat16)
            nc.sync.dma_start(out=ones_t[:], in_=ones.ap())
            locid_t = constp.tile([P, t_total], dt.float32)
            nc.sync.dma_start(out=locid_t[:], in_=locid.ap())
            locid_row_t = constp.tile([P, roww], dt.bfloat16)
            nc.sync.dma_start(out=locid_row_t[:], in_=locid_row.ap())
            invc_t = constp.tile([P, NC_CHUNKS], dt.float32)
            nc.sync.dma_start(out=invc_t[:], in_=invc.ap())

            # per-core partial community sums (bf16) and the mean table
            comm_sum = accp.tile([P, NC_CHUNKS * F], dt.bfloat16)
            mean_sb = accp.tile([P, NC_CHUNKS * F], dt.bfloat16)

            ar_bufs = []
            for h in range(4):
                ar_in = dramp.tile([P, QW], dt.bfloat16, name=f"ar_in{h}")
                ar_out = dramp.tile([P, QW], dt.bfloat16, name=f"ar_out{h}")
                ar_bufs.append((ar_in, ar_out))

            def emit_ar_quarter(h):
                ar_in, ar_out = ar_bufs[h]
                o = h * QW
                nc.sync.dma_start(out=ar_in, in_=comm_sum[:, o : o + QW])
                if use_collective:
                    nc.gpsimd.collective_compute(
                        "AllReduce",
                        mybir.AluOpType.add,
                        replica_groups=[list(range(M))],
                        ins=[ar_in.opt()],
                        outs=[ar_out.opt()],
                    )
                else:
                    nc.sync.dma_start(out=ar_out, in_=ar_in)
                nc.sync.dma_start(out=mean_sb[:, o : o + QW], in_=ar_out)
                for k in range(2 * h, 2 * h + 2):
                    nc.vector.tensor_scalar(
                        mean_sb[:, k * F : (k + 1) * F],
                        mean_sb[:, k * F : (k + 1) * F],
                        invc_t[:, k : k + 1],
                        None,
                        mybir.AluOpType.mult,
                    )

            # ---- phase-2 emission: B^T build stream + matmul stream ----
            outsb = [None]
            bt_aps = {}

            def emit_bt_build(q):
                bpart = (q % 3) * 32
                c0 = (q // 3) * (G * P)
                bc = psB.tile([P, G * P], dt.float32, tag="bc")
                nc.tensor.matmul(
                    bc[:],
                    lhsT=ones_t[bpart : bpart + 1, :],
                    rhs=locid_row_t[bpart : bpart + 1, c0 : c0 + G * P],
                    start=True,
                    stop=True,
                )
                btq = btp.tile([P, G * P], dt.bfloat16, tag="bt", name="btq")
                nc.vector.tensor_scalar(
                    btq[:],
                    bc[:],
                    iota_col_t[:, 0:1],
                    None,
                    mybir.AluOpType.is_equal,
                )
                bt_aps[q] = btq

            def emit_mm_part(q):
                t0 = q * G
                btq = bt_aps.pop(q)
                po = None
                for j2 in range(G):
                    t = t0 + j2
                    k = chunk_of_tile[t]
                    jb = t % BT
                    if jb == 0:
                        outsb[0] = outp.tile(
                            [P, BT * F], dt.bfloat16, tag="osb", name="osb"
                        )
                    if j2 % 2 == 0:
                        po = psO.tile([P, 2 * F], dt.float32, tag="po")
                    nc.tensor.matmul(
                        po[:, (j2 % 2) * F : (j2 % 2 + 1) * F],
                        lhsT=btq[:, j2 * P : (j2 + 1) * P],
                        rhs=mean_sb[:, k * F : (k + 1) * F],
                        start=True,
                        stop=True,
                    )
                    if j2 % 2 == 1:
                        if j2 == 1 and q % 2 == 1:
                            nc.vector.tensor_copy(
                                out=outsb[0][:, (jb - 1) * F : (jb + 1) * F],
                                in_=po[:],
                            )
                        else:
                            nc.scalar.copy(
                                out=outsb[0][:, (jb - 1) * F : (jb + 1) * F],
                                in_=po[:],
                            )
                    if jb == BT - 1:
                        b = t // BT
                        nc.sync.dma_start(
                            out=out_view[b], in_=outsb[0][:]
                        )

            # ---- main emission loop: phase 1 with interleaved phase 2 ----
            xsb = None
            ps = None
            b4 = None
            bt_built = 0
            mm_done = 0
            ar_emit_tile = {}

            for t in range(t_total):
                if t % BT == 0:
                    b = t // BT
                    xsb = xsp.tile([P, BT * F], dt.bfloat16, tag="xsb")
                    nc.sync.dma_start(out=xsb[:], in_=xs_view[b])
                if t % G == 0:
                    # batched one-hot build for G phase-1 tiles in one DVE op
                    b4 = bp.tile([P, G * P], dt.bfloat16, tag="b")
                    nc.vector.tensor_tensor(
                        out=b4[:].rearrange("p (g j) -> p g j", g=G),
                        in0=iota4_t[:].rearrange("p (g j) -> p g j", g=G),
                        in1=locid_t[:, t : t + G]
                        .unsqueeze(2)
                        .broadcast_to([P, G, P]),
                        op=mybir.AluOpType.is_equal,
                    )
                k = chunk_of_tile[t]
                first = t == tile_off[k]
                last = t == tile_off[k + 1] - 1
                if first:
                    ps = ps1.tile([P, F], dt.float32, tag="ps")
                j = t % BT
                jg = t % G
                nc.tensor.matmul(
                    ps[:],
                    lhsT=b4[:, jg * P : (jg + 1) * P],
                    rhs=xsb[:, j * F : (j + 1) * F],
                    start=first,
                    stop=last,
                )
                if last:
                    nc.vector.tensor_copy(
                        out=comm_sum[:, k * F : (k + 1) * F], in_=ps[:]
                    )
                    if k % 2 == 1:
                        emit_ar_quarter(k // 2)
                        ar_emit_tile[k // 2] = t
                # phase-2 streams, paced 1-per-G phase-1 tiles:
                # B^T builds have no AR dependency and run ahead; the
                # matmul+copy+store stream is gated on its AR half
                if t % G == G - 1:
                    if bt_built < ngrp and bt_built < mm_done + RUNAHEAD:
                        emit_bt_build(bt_built)
                        bt_built += 1
                    if mm_done < bt_built:
                        h = chunk_of_tile[mm_done * G + G - 1] // 2
                        if h in ar_emit_tile and t >= ar_emit_tile[h] + LAG_TILES:
                            emit_mm_part(mm_done)
                            mm_done += 1

            # tail: alternate remaining builds and matmul parts
            while mm_done < ngrp:
                if bt_built < ngrp and bt_built < mm_done + RUNAHEAD:
                    emit_bt_build(bt_built)
                    bt_built += 1
                else:
                    emit_mm_part(mm_done)
                    mm_done += 1

    nc.compile()
    return nc


def kernel(x, community):
    global LAST_RESULTS
    from concourse.bass_utils import run_bass_kernel_spmd

    in_maps, plan = _host_prep(x, community)
    nc = _build_program(plan)
    res = run_bass_kernel_spmd(nc, in_maps, core_ids=list(range(M)))
    LAST_RESULTS = res
    nl = plan["nl"]
    t_total = plan["t_total"]
    nblk = t_total // BT
    outs = []
    for m in range(M):
        out_blk = np.asarray(res.results[m]["out"])  # [nblk*P, BT*F] bf16
        out_sorted = (
            out_blk.reshape(nblk, P, BT, F)
            .transpose(0, 2, 1, 3)
            .reshape(t_total * P, F)
        )
        orig = plan["origs"][m]
        valid = orig >= 0
        out_m = np.empty((nl, F), dtype=np.float32)
        out_m[orig[valid]] = out_sorted[valid]
        outs.append(out_m)
    return np.concatenate(outs, axis=0)


# revision 13
# speedup vs baseline: 1.1352x; 1.0272x over previous
"""Trainium2 Bass kernel for CommunityPassing (segment mean + gather).

Algorithm (8 NeuronCores, data-parallel over nodes):
  host: shard x/community over 8 cores along the node axis; within each
        shard, stably sort node indices by community id and pack them into
        128-row tiles grouped by community "chunk" (128 communities per
        chunk, 8 chunks for 1000 communities). Pad each (core, chunk)
        block to a shared tile count so all cores run one SPMD program.
        Cast x to bf16 and lay tiles out in 16-tile blocks with
        per-partition-contiguous 8KB rows for full-rate DMA.
  dev:  phase 1 - stream sorted x blocks; per tile build a one-hot
        B[node, local_comm] with a DVE is_equal; matmul B^T @ x_tile
        accumulating in PSUM per community chunk -> per-core partial sums
        (bf16).  AllReduce the sums in four 128KB quarters (chunk pairs)
        so the means arrive progressively and the serialized collective
        chain overlaps the phase-2 stream despite ~40us core-launch skew.
        Scale by host-computed 1/count -> community-mean table (bf16).
        phase 2 - per 4-tile group, broadcast the group's local ids
        across partitions with a K=1 matmul (ones^T @ locid_row), build
        B^T directly with one is_equal against the partition index, then
        per tile matmul (lhsT=B^T) @ mean_chunk -> out tile.  The B^T
        builds have no collective dependency and run ahead of the gated
        matmul+copy+store stream (RUNAHEAD ring).  PSUM->SBUF copies are
        split between the Scalar and Vector engines; 16-tile 1MB blocks
        are stored with per-partition-contiguous rows.
  host: unpack blocks, scatter rows back to original order, cast to f32.
"""

import os
import sys

import numpy as np

for _p in ("/opt/trn_rl_repo", "/opt/pypackages"):
    if _p not in sys.path and os.path.isdir(_p):
        sys.path.append(_p)

# Problem constants (hardcoded per the task contract).
N_FULL = 500000
F = 256
NUM_COMMS = 1000
EPS = 1e-12
M = 8               # cores
P = 128             # partitions
NC_CHUNKS = 8       # community chunks of 128 (8*128 = 1024 >= 1000)
BT = 16             # tiles per x/out DMA block (8KB per partition = 1MB)
G = 4               # tiles per phase-2 B^T build group
LAG_TILES = 150  # covers ~40us core-launch skew + AR latency
RUNAHEAD = 28    # B^T build groups allowed ahead of the matmul stream     # phase-1 tiles between AR issue and first dependent p2 op

# Stash of the most recent run's BassKernelResults (for test harnesses).
LAST_RESULTS = None


def _host_prep(x, community):
    """Build per-core device inputs. Returns (in_maps, plan)."""
    import ml_dtypes

    bf16 = ml_dtypes.bfloat16
    x = np.ascontiguousarray(np.asarray(x, dtype=np.float32))
    community = np.asarray(community).astype(np.int64)
    n = x.shape[0]
    assert n % M == 0
    nl = n // M

    comm_sh = community.reshape(M, nl)
    perms = np.argsort(comm_sh, axis=1, kind="stable")
    comm_sorted = np.take_along_axis(comm_sh, perms, axis=1)

    # per (core, chunk) node counts
    chunk_ids = comm_sorted >> 7  # // 128
    cnts = np.zeros((M, NC_CHUNKS), dtype=np.int64)
    for m in range(M):
        bc = np.bincount(chunk_ids[m], minlength=NC_CHUNKS)
        cnts[m] = bc[:NC_CHUNKS]
    t_k = np.maximum(1, -(-cnts.max(axis=0) // P))  # ceil, shared by all cores
    # pad the total tile count to a multiple of BT (extend the last chunk)
    t_total = int(t_k.sum())
    pad = (-t_total) % BT
    t_k[-1] += pad
    t_total += pad
    chunk_of_tile = np.repeat(np.arange(NC_CHUNKS), t_k)
    tile_off = np.concatenate([[0], np.cumsum(t_k)])  # tile index base per chunk

    # counts -> 1/max(cnt, eps), [p, k] layout (community id = k*128 + p)
    cnt_full = np.bincount(community, minlength=NUM_COMMS).astype(np.float32)
    inv_pad = np.zeros((NC_CHUNKS * P,), np.float32)
    inv_pad[:NUM_COMMS] = 1.0 / np.maximum(cnt_full, np.float32(EPS))
    invc = np.ascontiguousarray(inv_pad.reshape(NC_CHUNKS, P).T)  # [128, 8]

    iota_t = np.ascontiguousarray(
        np.tile(np.arange(P, dtype=np.float32), (P, 1))
    ).astype(bf16)  # [128, 128], each row 0..127
    iota_col = np.arange(P, dtype=np.float32).reshape(P, 1)  # [128, 1]
    ones_t = np.ones((P, P), dtype=bf16)
    iota4 = np.ascontiguousarray(np.tile(iota_t, (1, G)))  # [128, G*128]

    nblk = t_total // BT
    ngrp = t_total // G
    roww = -(-ngrp // 3) * (G * P)  # locid_row cols per partition-slot

    in_maps = []
    origs = []
    for m in range(M):
        x_m = x[m * nl : (m + 1) * nl]
        xs = np.zeros((t_total * P, F), dtype=bf16)
        locid = np.full((t_total * P,), -1.0, dtype=np.float32)
        orig = np.full((t_total * P,), -1, dtype=np.int64)
        start = 0
        for k in range(NC_CHUNKS):
            c = int(cnts[m, k])
            row = int(tile_off[k]) * P
            sel = perms[m, start : start + c]
            xs[row : row + c] = x_m[sel]
            orig[row : row + c] = sel
            locid[row : row + c] = comm_sorted[m, start : start + c] - k * P
            start += c
        origs.append(orig)

        # blocked x layout: [nblk, P, BT*F] with 8KB contiguous per partition
        xs_blk = np.ascontiguousarray(
            xs.reshape(nblk, BT, P, F).transpose(0, 2, 1, 3)
        ).reshape(nblk * P, BT * F)
        # per-tile local ids, [128, T] (partition = node-in-tile)
        locid_t = np.ascontiguousarray(locid.reshape(t_total, P).T)
        # groups of G tiles packed at partitions {0,32,64,96} for the
        # K=1 broadcast matmul (tile_position constraint)
        locid_row = np.zeros((P, roww), dtype=bf16)
        lr = locid.reshape(ngrp, G * P).astype(bf16)
        for q in range(ngrp):
            part = (q % 3) * 32
            c0 = (q // 3) * (G * P)
            locid_row[part, c0 : c0 + G * P] = lr[q]

        in_maps.append(
            {
                "xs": xs_blk,
                "locid": locid_t,
                "locid_row": locid_row,
                "iota": iota_t,
                "iota4": iota4,
                "iota_col": iota_col,
                "ones": ones_t,
                "invc": invc,
            }
        )

    plan = {
        "nl": nl,
        "t_total": t_total,
        "chunk_of_tile": [int(v) for v in chunk_of_tile],
        "tile_off": [int(v) for v in tile_off],
        "origs": origs,
        "roww": roww,
    }
    return in_maps, plan


def _build_program(plan, use_collective=True):
    from concourse import bacc, mybir, tile

    t_total = plan["t_total"]
    chunk_of_tile = plan["chunk_of_tile"]
    tile_off = plan["tile_off"]
    roww = plan["roww"]
    nblk = t_total // BT
    ngrp = t_total // G
    QW = 2 * F  # 2 chunks of the community table per AR quarter

    dt = mybir.dt
    nc = bacc.Bacc("TRN2", target_bir_lowering=False, debug=False, num_devices=M)

    xs = nc.dram_tensor("xs", [nblk * P, BT * F], dt.bfloat16, kind="ExternalInput")
    locid = nc.dram_tensor("locid", [P, t_total], dt.float32, kind="ExternalInput")
    locid_row = nc.dram_tensor(
        "locid_row", [P, roww], dt.bfloat16, kind="ExternalInput"
    )
    iota = nc.dram_tensor("iota", [P, P], dt.bfloat16, kind="ExternalInput")
    iota4 = nc.dram_tensor("iota4", [P, G * P], dt.bfloat16, kind="ExternalInput")
    iota_col = nc.dram_tensor("iota_col", [P, 1], dt.float32, kind="ExternalInput")
    ones = nc.dram_tensor("ones", [P, P], dt.bfloat16, kind="ExternalInput")
    invc = nc.dram_tensor("invc", [P, NC_CHUNKS], dt.float32, kind="ExternalInput")
    out = nc.dram_tensor("out", [nblk * P, BT * F], dt.bfloat16, kind="ExternalOutput")

    xs_view = xs.ap().rearrange("(b p) w -> b p w", p=P)  # [nblk, 128, BT*F]
    out_view = out.ap().rearrange("(b p) w -> b p w", p=P)

    with tile.TileContext(nc) as tc:
        with (
            tc.tile_pool(name="const", bufs=1) as constp,
            tc.tile_pool(name="acc", bufs=1) as accp,
            tc.tile_pool(name="xsp", bufs=3) as xsp,
            tc.tile_pool(name="bp", bufs=6) as bp,
            tc.tile_pool(name="btp", bufs=30) as btp,
            tc.tile_pool(name="outp", bufs=3) as outp,
            tc.tile_pool(name="ps1", bufs=2, space="PSUM") as ps1,
            tc.tile_pool(name="psB", bufs=2, space="PSUM") as psB,
            tc.tile_pool(name="psO", bufs=3, space="PSUM") as psO,
            tc.tile_pool(name="dram", bufs=1, space="DRAM") as dramp,
        ):
            iota_t = constp.tile([P, P], dt.bfloat16)
            nc.sync.dma_start(out=iota_t[:], in_=iota.ap())
            iota4_t = constp.tile([P, G * P], dt.bfloat16)
            nc.sync.dma_start(out=iota4_t[:], in_=iota4.ap())
            iota_col_t = constp.tile([P, 1], dt.float32)
            nc.sync.dma_start(out=iota_col_t[:], in_=iota_col.ap())
            ones_t = constp.tile([P, P], dt.bfloat16)
            nc.sync.dma_start(out=ones_t[:], in_=ones.ap())
            locid_t = constp.tile([P, t_total], dt.float32)
            nc.sync.dma_start(out=locid_t[:], in_=locid.ap())
            locid_row_t = constp.tile([P, roww], dt.bfloat16)
            nc.sync.dma_start(out=locid_row_t[:], in_=locid_row.ap())
            invc_t = constp.tile([P, NC_CHUNKS], dt.float32)
            nc.sync.dma_start(out=invc_t[:], in_=invc.ap())

            # per-core partial community sums (bf16) and the mean table
            comm_sum = accp.tile([P, NC_CHUNKS * F], dt.bfloat16)
            mean_sb = accp.tile([P, NC_CHUNKS * F], dt.bfloat16)

            ar_bufs = []
            for h in range(4):
                ar_in = dramp.tile([P, QW], dt.bfloat16, name=f"ar_in{h}")
                ar_out = dramp.tile([P, QW], dt.bfloat16, name=f"ar_out{h}")
                ar_bufs.append((ar_in, ar_out))

            def emit_ar_quarter(h):
                ar_in, ar_out = ar_bufs[h]
                o = h * QW
                nc.sync.dma_start(out=ar_in, in_=comm_sum[:, o : o + QW])
                if use_collective:
                    nc.gpsimd.collective_compute(
                        "AllReduce",
                        mybir.AluOpType.add,
                        replica_groups=[list(range(M))],
                        ins=[ar_in.opt()],
                        outs=[ar_out.opt()],
                    )
                else:
                    nc.sync.dma_start(out=ar_out, in_=ar_in)
                nc.sync.dma_start(out=mean_sb[:, o : o + QW], in_=ar_out)
                for k in range(2 * h, 2 * h + 2):
                    nc.vector.tensor_scalar(
                        mean_sb[:, k * F : (k + 1) * F],
                        mean_sb[:, k * F : (k + 1) * F],
                        invc_t[:, k : k + 1],
                        None,
                        mybir.AluOpType.mult,
                    )

            # ---- phase-2 emission: B^T build stream + matmul stream ----
            outsb = [None]
            bt_aps = {}

            def emit_bt_build(q):
                bpart = (q % 3) * 32
                c0 = (q // 3) * (G * P)
                bc = psB.tile([P, G * P], dt.float32, tag="bc")
                nc.tensor.matmul(
                    bc[:],
                    lhsT=ones_t[bpart : bpart + 1, :],
                    rhs=locid_row_t[bpart : bpart + 1, c0 : c0 + G * P],
                    start=True,
                    stop=True,
                )
                btq = btp.tile([P, G * P], dt.bfloat16, tag="bt", name="btq")
                nc.vector.tensor_scalar(
                    btq[:],
                    bc[:],
                    iota_col_t[:, 0:1],
                    None,
                    mybir.AluOpType.is_equal,
                )
                bt_aps[q] = btq

            def emit_mm_part(q):
                t0 = q * G
                btq = bt_aps.pop(q)
                po = None
                for j2 in range(G):
                    t = t0 + j2
                    k = chunk_of_tile[t]
                    jb = t % BT
                    if jb == 0:
                        outsb[0] = outp.tile(
                            [P, BT * F], dt.bfloat16, tag="osb", name="osb"
                        )
                    if j2 % 2 == 0:
                        po = psO.tile([P, 2 * F], dt.float32, tag="po")
                    nc.tensor.matmul(
                        po[:, (j2 % 2) * F : (j2 % 2 + 1) * F],
                        lhsT=btq[:, j2 * P : (j2 + 1) * P],
                        rhs=mean_sb[:, k * F : (k + 1) * F],
                        start=True,
                        stop=True,
                    )
                    if j2 % 2 == 1:
                        if j2 == 1 and q % 2 == 1:
                            nc.vector.tensor_copy(
                                out=outsb[0][:, (jb - 1) * F : (jb + 1) * F],
                                in_=po[:],
                            )
                        else:
                            nc.scalar.copy(
                                out=outsb[0][:, (jb - 1) * F : (jb + 1) * F],
                                in_=po[:],
                            )
                    if jb == BT - 1:
                        b = t // BT
                        nc.sync.dma_start(
                            out=out_view[b], in_=outsb[0][:]
                        )

            # ---- main emission loop: phase 1 with interleaved phase 2 ----
            xsb = None
            ps = None
            b4 = None
            bt_built = 0
            mm_done = 0
            ar_emit_tile = {}

            for t in range(t_total):
                if t % BT == 0:
                    b = t // BT
                    xsb = xsp.tile([P, BT * F], dt.bfloat16, tag="xsb")
                    nc.sync.dma_start(out=xsb[:], in_=xs_view[b])
                if t % G == 0:
                    # batched one-hot build for G phase-1 tiles in one DVE op
                    b4 = bp.tile([P, G * P], dt.bfloat16, tag="b")
                    nc.vector.tensor_tensor(
                        out=b4[:].rearrange("p (g j) -> p g j", g=G),
                        in0=iota4_t[:].rearrange("p (g j) -> p g j", g=G),
                        in1=locid_t[:, t : t + G]
                        .unsqueeze(2)
                        .broadcast_to([P, G, P]),
                        op=mybir.AluOpType.is_equal,
                    )
                k = chunk_of_tile[t]
                first = t == tile_off[k]
                last = t == tile_off[k + 1] - 1
                if first:
                    ps = ps1.tile([P, F], dt.float32, tag="ps")
                j = t % BT
                jg = t % G
                nc.tensor.matmul(
                    ps[:],
                    lhsT=b4[:, jg * P : (jg + 1) * P],
                    rhs=xsb[:, j * F : (j + 1) * F],
                    start=first,
                    stop=last,
                )
                if last:
                    nc.vector.tensor_copy(
                        out=comm_sum[:, k * F : (k + 1) * F], in_=ps[:]
                    )
                    if k % 2 == 1:
                        emit_ar_quarter(k // 2)
                        ar_emit_tile[k // 2] = t
                # phase-2 streams, paced 1-per-G phase-1 tiles:
                # B^T builds have no AR dependency and run ahead; the
                # matmul+copy+store stream is gated on its AR half
                if t % G == G - 1:
                    if bt_built < ngrp and bt_built < mm_done + RUNAHEAD:
                        emit_bt_build(bt_built)
                        bt_built += 1
                    if mm_done < bt_built:
                        h = chunk_of_tile[mm_done * G + G - 1] // 2
                        if h in ar_emit_tile and t >= ar_emit_tile[h] + LAG_TILES:
                            emit_mm_part(mm_done)
                            mm_done += 1

            # tail: alternate remaining builds and matmul parts
            while mm_done < ngrp:
                if bt_built < ngrp and bt_built < mm_done + RUNAHEAD:
                    emit_bt_build(bt_built)
                    bt_built += 1
                else:
                    emit_mm_part(mm_done)
                    mm_done += 1

    nc.compile()
    return nc


def kernel(x, community):
    global LAST_RESULTS
    from concourse.bass_utils import run_bass_kernel_spmd

    in_maps, plan = _host_prep(x, community)
    nc = _build_program(plan)
    res = run_bass_kernel_spmd(nc, in_maps, core_ids=list(range(M)))
    LAST_RESULTS = res
    nl = plan["nl"]
    t_total = plan["t_total"]
    nblk = t_total // BT
    outs = []
    for m in range(M):
        out_blk = np.asarray(res.results[m]["out"])  # [nblk*P, BT*F] bf16
        out_sorted = (
            out_blk.reshape(nblk, P, BT, F)
            .transpose(0, 2, 1, 3)
            .reshape(t_total * P, F)
        )
        orig = plan["origs"][m]
        valid = orig >= 0
        out_m = np.empty((nl, F), dtype=np.float32)
        out_m[orig[valid]] = out_sorted[valid]
        outs.append(out_m)
    return np.concatenate(outs, axis=0)
